# revision 25
# baseline (speedup 1.0000x reference)
"""Trainium2 Bass kernel for nn_Aggregator (gnn_message_passing).

Computes, for FULL inputs:
    weighted = embeds_stack * rel_weights[:, None]
    sums     = segment_sum(weighted, segment_ids, num_segments=5120)
    mean     = sums / length[:, None]
    h        = relu(mean @ W.T + b)                        # [5120, 256]
    out      = concat([h.reshape(512, 10, 256),
                       broadcast(ent_embeds[s_tem])], -1)  # [512, 10, 384]

segment_ids is repeat(arange(5120), 200): groups are 200 contiguous rows,
and 1024000/8 = 640 complete groups per core, so the 8-way shard along the
neighbor axis needs no cross-core reduction at all.  Each core processes
640 groups (= 64 batches x 10 timesteps) end to end and produces its own
[640, 384] slice of the output.

Device strategy per core (memory-bound; ~131 MB of embeds per core):
  Phase 1: for each block of 128 groups, stream [128, 20, 256] tiles
    (2.56 MB DMA, 20 KB contiguous per partition), multiply in place by
    the per-(group, neighbor) weight (pre-divided by group length on the
    host) on the Vector engine with float32r-rounded output, and
    accumulate neighbor-row pairs into a [128, 512] PSUM bank with
    identity-stationary float32r matmuls (PSUM += [tile_i | tile_i+1]).
    The two 256-wide halves are summed when the block finishes.
  Phase 2 (per block, overlapped with the next block's streaming):
    PE-transpose the [128 groups, 256] means to feature-major, matmul
    with host-pretransposed W (K=256 on partitions), bias + relu on the
    Scalar engine, transpose back, DMA to out[:, 0:256].
  Ent part: indirect-DMA gather of the 64 s_tem rows from the replicated
    ent_embeds table, broadcast-DMA into out[:, 256:384].
"""

import os

os.environ.setdefault("JAX_PLATFORMS", "axon")

from contextlib import ExitStack

import numpy as np

import concourse.bass as bass
import concourse.tile as tile
from concourse import mybir
from concourse import bass_utils

# Problem geometry (hardcoded; matches reference.setup_inputs()).
H2 = 256          # 2 * H_DIM
BATCH = 512
SEQ = 10
NI = 200          # neighbors per group
NUM_ENTS = 50000
H = 128
OUT_F = H2 + H    # 384
P = 128
NCORES = 8
NB = 5            # group blocks of 128 per core
GC = NB * P       # 640 groups per core
BC = GC // SEQ    # 64 batches per core
KI = 20           # neighbor rows per DMA tile (20 KB contiguous/partition)

F32 = mybir.dt.float32
F32R = mybir.dt.float32r


def _split_multi_waits(nc):
    """Split instructions carrying >1 sem wait into single-wait NoOps.

    The walrus build in this container rejects any instruction with more
    than one sync wait command, but Tile's semaphore assignment freely
    attaches several waits to one instruction (and every outstanding
    DMA-queue sem to the tail drain).  Insert same-engine NoOps, each
    carrying one of the extra waits, immediately before such instructions;
    sequencer program order makes this semantically identical.
    """
    uid = [0]
    for f in nc.m.functions:
        for bb in f.blocks:
            insts = list(bb.instructions)
            out, changed = [], False
            for inst in insts:
                si = inst.sync_info
                if si is not None and si.on_wait and len(si.on_wait) > 1:
                    waits = list(si.on_wait)
                    for w in waits[:-1]:
                        nop = mybir.InstNoOp(
                            name=f"I-waitsplit-{uid[0]}", ins=[], outs=[])
                        uid[0] += 1
                        nop.engine = inst.engine
                        nop.sync_info = mybir.SyncInfo(
                            on_wait=[w], on_update=[])
                        out.append(nop)
                    si.on_wait = [waits[-1]]
                    changed = True
                out.append(inst)
            if changed:
                bb.instructions = out


def build_program(nch=NI // KI):
    """Build the per-core Bass program.  nch = neighbor chunks per block
    (nch * KI neighbors per group; 10 for the full problem)."""
    ni = nch * KI
    assert KI % 2 == 0
    nc = bass.Bass("TRN2", target_bir_lowering=False, debug=False, num_devices=1)

    emb = nc.dram_tensor("emb", [GC * ni, H2], F32, kind="ExternalInput").ap()
    # host-prepped dense layouts (partition-major)
    wre = nc.dram_tensor("wre", [P, NB, ni], F32, kind="ExternalInput").ap()
    wt = nc.dram_tensor("wt", [P, 2, H2], F32, kind="ExternalInput").ap()
    bvec = nc.dram_tensor("bvec", [P, 2], F32, kind="ExternalInput").ap()
    ident = nc.dram_tensor("ident", [P, P], F32, kind="ExternalInput").ap()
    ents = nc.dram_tensor("ents", [NUM_ENTS, H], F32, kind="ExternalInput").ap()
    stem = nc.dram_tensor("stem", [BC, 1], mybir.dt.int32, kind="ExternalInput").ap()
    out = nc.dram_tensor("out", [GC, OUT_F], F32, kind="ExternalOutput").ap()

    # DRAM views
    e4 = emb.rearrange("(b p n) f -> b p n f", b=NB, p=P)        # [NB,128,ni,256]
    out_bp = out.rearrange("(b p) o -> b p o", b=NB)             # [NB,128,384]
    out_bt = out.rearrange("(bt s) o -> bt s o", s=SEQ)          # [64,10,384]

    with tile.TileContext(nc) as tc:
        with ExitStack() as ctx:
            consts = ctx.enter_context(tc.tile_pool(name="consts", bufs=1))
            ebuf = ctx.enter_context(tc.tile_pool(name="ebuf", bufs=6))
            pbuf = ctx.enter_context(tc.tile_pool(name="pbuf", bufs=6))
            small = ctx.enter_context(tc.tile_pool(name="small", bufs=1))
            blk = ctx.enter_context(tc.tile_pool(name="blk", bufs=2))
            psum_acc = ctx.enter_context(
                tc.tile_pool(name="psum_acc", bufs=2, space="PSUM"))
            psum_tr = ctx.enter_context(
                tc.tile_pool(name="psum_tr", bufs=2, space="PSUM"))
            psum_h = ctx.enter_context(
                tc.tile_pool(name="psum_h", bufs=2, space="PSUM"))

            # ---- pre-issue the first input DMAs so HBM streaming starts
            # before the consts issue/land ----
            pre = []
            for c in range(min(3, nch)):
                et0 = ebuf.tile([P, KI, H2], F32, name="et")
                nc.sync.dma_start(et0[:], e4[0, :, c * KI:(c + 1) * KI, :])
                pre.append(et0)

            # ---- constants into SBUF (all dense, partition-major) ----
            ident_sb = consts.tile([P, P], F32)
            nc.sync.dma_start(ident_sb[:], ident[:, :])
            identr_sb = consts.tile([P, P], F32R)
            nc.vector.tensor_copy(identr_sb[:], ident_sb[:])
            wre_sb = consts.tile([P, NB, ni], F32)
            nc.sync.dma_start(wre_sb[:], wre[:, :, :])
            wt_sb = consts.tile([P, 2, H2], F32)
            nc.sync.dma_start(wt_sb[:], wt[:, :, :])
            wtr_sb = consts.tile([P, 2, H2], F32R)
            nc.vector.tensor_copy(wtr_sb[:], wt_sb[:])
            bias_sb = consts.tile([P, 2], F32)
            nc.sync.dma_start(bias_sb[:], bvec[:, :])
            stem_sb = consts.tile([BC, 1], mybir.dt.int32)
            nc.sync.dma_start(stem_sb[:], stem[:, :])

            # ---- phase 1 + per-block phase 2 ----
            for b in range(NB):
                acc = psum_acc.tile([P, 2 * H2], F32, name="acc")
                if ni == NI and b == NB - 1:
                    # taper the final chunks so the compute chain exposed
                    # after the very last DMA is short
                    chunk_list = [(k * KI, KI) for k in range(9)] + \
                                 [(9 * KI + k * 4, 4) for k in range(5)]
                else:
                    chunk_list = [(k * KI, KI) for k in range(nch)]
                n_chunks = len(chunk_list)
                for ci, (r0, nr) in enumerate(chunk_list):
                    if b == 0 and ci < len(pre):
                        et = pre[ci]
                    else:
                        et = ebuf.tile([P, nr, H2], F32, name="et")
                        nc.sync.dma_start(et[:], e4[b, :, r0:r0 + nr, :])
                    for i in range(0, nr, 2):
                        pair = pbuf.tile([P, 2, H2], F32R, name="pair")
                        for j in range(2):
                            k = r0 + i + j
                            nc.vector.tensor_scalar_mul(
                                pair[:, j, :], et[:, i + j, :],
                                wre_sb[:, b, k:k + 1],
                            )
                        nc.tensor.matmul(
                            acc[:],
                            lhsT=identr_sb[:],
                            rhs=pair[:, :, :],
                            start=(ci == 0 and i == 0),
                            stop=(ci == n_chunks - 1 and i == nr - 2),
                        )
                # fold the two halves: mean for this block [128, 256]
                mean_b = blk.tile([P, H2], F32, name="mean_b")
                nc.scalar.copy(mean_b[:], acc[:, 0:H2])
                nc.vector.tensor_add(mean_b[:], mean_b[:], acc[:, H2:2 * H2])

                # transpose to feature-major (float32r for the W matmul)
                meanT_b = blk.tile([P, 2, P], F32R, name="meanT_b")
                for fh in range(2):
                    pt = psum_tr.tile([P, P], F32, name="pt")
                    nc.tensor.transpose(
                        pt[:], mean_b[:, fh * P:(fh + 1) * P], ident_sb[:])
                    nc.scalar.copy(meanT_b[:, fh, :], pt[:])

                # hT = relu(WT.T @ meanT + bias)
                hT_b = blk.tile([P, 2, P], F32, name="hT_b")
                for oh in range(2):
                    ph = psum_h.tile([P, P], F32, name="ph")
                    for fh in range(2):
                        nc.tensor.matmul(
                            ph[:],
                            lhsT=wtr_sb[:, fh, oh * P:(oh + 1) * P],
                            rhs=meanT_b[:, fh, :],
                            start=(fh == 0),
                            stop=(fh == 1),
                        )
                    nc.scalar.activation(
                        hT_b[:, oh, :], ph[:],
                        func=mybir.ActivationFunctionType.Relu,
                        bias=bias_sb[:, oh:oh + 1],
                        scale=1.0,
                    )

                # back to group-major rows, store
                h_b = blk.tile([P, H2], F32, name="h_b")
                for oh in range(2):
                    pt2 = psum_tr.tile([P, P], F32, name="pt2")
                    nc.tensor.transpose(pt2[:], hT_b[:, oh, :], ident_sb[:])
                    nc.scalar.copy(h_b[:, oh * P:(oh + 1) * P], pt2[:])
                nc.sync.dma_start(out_bp[b, :, 0:H2], h_b[:])

                if b == 2:
                    # ent gather + broadcast store, scheduled mid-stream so
                    # it fills a block-boundary gap instead of the tail
                    ent_sb = small.tile([BC, H], F32)
                    nc.gpsimd.indirect_dma_start(
                        out=ent_sb[:],
                        out_offset=None,
                        in_=ents[:, :],
                        in_offset=bass.IndirectOffsetOnAxis(
                            ap=stem_sb[:, :1], axis=0),
                    )
                    ent_bc = bass.AP(
                        ent_sb.tensor, ent_sb.offset,
                        [ent_sb.ap[0], [0, SEQ], ent_sb.ap[1]],
                    )
                    nc.sync.dma_start(out_bt[:, :, H2:OUT_F], ent_bc)

    _split_multi_waits(nc)
    return nc


_PROGRAM_CACHE = {}


def _get_program(nch):
    if nch not in _PROGRAM_CACHE:
        _PROGRAM_CACHE[nch] = build_program(nch)
    return _PROGRAM_CACHE[nch]


def make_in_maps(embeds_stack, rel_weights, ent_embeds, W, b, length, s_tem,
                 nch=NI // KI):
    ni = nch * KI
    tot_c = GC * ni  # rows per core
    emb = np.ascontiguousarray(np.asarray(embeds_stack, dtype=np.float32))
    relw = np.asarray(rel_weights, dtype=np.float32)
    length = np.asarray(length, dtype=np.float32).reshape(NCORES * GC, 1)
    wmat = np.asarray(W, dtype=np.float32)
    bv = np.asarray(b, dtype=np.float32)
    ents = np.ascontiguousarray(np.asarray(ent_embeds, dtype=np.float32))
    stem = np.asarray(s_tem, dtype=np.int32)

    # wt[p, fh, o] = W[o, fh*128 + p]  (dense partition-major W^T chunks)
    wt = np.ascontiguousarray(
        wmat.T.reshape(2, P, H2).transpose(1, 0, 2))
    bvec = np.ascontiguousarray(bv.reshape(2, P).T)
    ident = np.eye(P, dtype=np.float32)

    # weight / length, laid out [128, NB, ni] per core (partition-major)
    wre_all = (relw.reshape(NCORES * GC, ni) / length).astype(np.float32)
    wre_all = wre_all.reshape(NCORES, NB, P, ni)

    in_maps = []
    for c in range(NCORES):
        in_maps.append({
            "emb": np.ascontiguousarray(emb[c * tot_c:(c + 1) * tot_c]),
            "wre": np.ascontiguousarray(wre_all[c].transpose(1, 0, 2)),
            "wt": wt,
            "bvec": bvec,
            "ident": ident,
            "ents": ents,
            "stem": np.ascontiguousarray(stem[c * BC:(c + 1) * BC].reshape(BC, 1)),
        })
    return in_maps


def run_sharded(inputs, nch=NI // KI, trace=False):
    """inputs: full-input dict as from reference.setup_inputs().
    Returns (full_output [512, 10, 384], BassKernelResults)."""
    nc = _get_program(nch)
    in_maps = make_in_maps(
        inputs["embeds_stack"], inputs["rel_weights"], inputs["ent_embeds"],
        inputs["W"], inputs["b"], inputs["length"], inputs["s_tem"], nch=nch,
    )
    last_err = None
    for attempt in range(3):
        try:
            res = bass_utils.run_bass_kernel_spmd(
                nc, in_maps, core_ids=list(range(NCORES)), trace=trace,
            )
            break
        except Exception as e:  # transient NRT_EXEC_UNIT_UNRECOVERABLE etc.
            last_err = e
            if attempt == 2:
                raise
            import time
            time.sleep(10)
    parts = [res.results[c]["out"].reshape(BC, SEQ, OUT_F) for c in range(NCORES)]
    full = np.concatenate(parts, axis=0)
    return full, res


def kernel(embeds_stack, rel_weights, ent_embeds, W, b,
           length, segment_ids, len_non_zero, s_tem):
    full, _ = run_sharded({
        "embeds_stack": embeds_stack, "rel_weights": rel_weights,
        "ent_embeds": ent_embeds, "W": W, "b": b, "length": length,
        "segment_ids": segment_ids, "len_non_zero": len_non_zero,
        "s_tem": s_tem,
    })
    return full


# revision 26
# speedup vs baseline: 1.1846x; 1.1846x over previous
"""Trainium2 Bass kernel for nn_Aggregator (gnn_message_passing).

Computes, for FULL inputs:
    weighted = embeds_stack * rel_weights[:, None]
    sums     = segment_sum(weighted, segment_ids, num_segments=5120)
    mean     = sums / length[:, None]
    h        = relu(mean @ W.T + b)                        # [5120, 256]
    out      = concat([h.reshape(512, 10, 256),
                       broadcast(ent_embeds[s_tem])], -1)  # [512, 10, 384]

segment_ids is repeat(arange(5120), 200): groups are 200 contiguous rows,
and 1024000/8 = 640 complete groups per core, so the 8-way shard along the
neighbor axis needs no cross-core reduction at all.  Each core processes
640 groups (= 64 batches x 10 timesteps) end to end and produces its own
[640, 384] slice of the output.

Device strategy per core (memory-bound; ~131 MB of embeds per core):
  Phase 1: for each block of 128 groups, stream [128, 20, 256] tiles
    (2.56 MB DMA, 20 KB contiguous per partition), multiply in place by
    the per-(group, neighbor) weight (pre-divided by group length on the
    host) on the Vector engine with float32r-rounded output, and
    accumulate neighbor-row pairs into a [128, 512] PSUM bank with
    identity-stationary float32r matmuls (PSUM += [tile_i | tile_i+1]).
    The two 256-wide halves are summed when the block finishes.
  Phase 2 (per block, overlapped with the next block's streaming):
    PE-transpose the [128 groups, 256] means to feature-major, matmul
    with host-pretransposed W (K=256 on partitions), bias + relu on the
    Scalar engine, transpose back, DMA to out[:, 0:256].
  Ent part: indirect-DMA gather of the 64 s_tem rows from the replicated
    ent_embeds table, broadcast-DMA into out[:, 256:384].
"""

import os

os.environ.setdefault("JAX_PLATFORMS", "axon")

from contextlib import ExitStack

import numpy as np

import concourse.bass as bass
import concourse.tile as tile
from concourse import mybir
from concourse import bass_utils

# Problem geometry (hardcoded; matches reference.setup_inputs()).
H2 = 256          # 2 * H_DIM
BATCH = 512
SEQ = 10
NI = 200          # neighbors per group
NUM_ENTS = 50000
H = 128
OUT_F = H2 + H    # 384
P = 128
NCORES = 8
NB = 5            # group blocks of 128 per core
GC = NB * P       # 640 groups per core
BC = GC // SEQ    # 64 batches per core
KI = 20           # neighbor rows per DMA tile (20 KB contiguous/partition)

F32 = mybir.dt.float32
F32R = mybir.dt.float32r


def _split_multi_waits(nc):
    """Split instructions carrying >1 sem wait into single-wait NoOps.

    The walrus build in this container rejects any instruction with more
    than one sync wait command, but Tile's semaphore assignment freely
    attaches several waits to one instruction (and every outstanding
    DMA-queue sem to the tail drain).  Insert same-engine NoOps, each
    carrying one of the extra waits, immediately before such instructions;
    sequencer program order makes this semantically identical.
    """
    uid = [0]
    for f in nc.m.functions:
        for bb in f.blocks:
            insts = list(bb.instructions)
            out, changed = [], False
            for inst in insts:
                si = inst.sync_info
                if si is not None and si.on_wait and len(si.on_wait) > 1:
                    waits = list(si.on_wait)
                    for w in waits[:-1]:
                        nop = mybir.InstNoOp(
                            name=f"I-waitsplit-{uid[0]}", ins=[], outs=[])
                        uid[0] += 1
                        nop.engine = inst.engine
                        nop.sync_info = mybir.SyncInfo(
                            on_wait=[w], on_update=[])
                        out.append(nop)
                    si.on_wait = [waits[-1]]
                    changed = True
                out.append(inst)
            if changed:
                bb.instructions = out


def build_program(nch=NI // KI):
    """Build the per-core Bass program.  nch = neighbor chunks per block
    (nch * KI neighbors per group; 10 for the full problem)."""
    ni = nch * KI
    assert KI % 2 == 0
    nc = bass.Bass("TRN2", target_bir_lowering=False, debug=False, num_devices=1)

    emb = nc.dram_tensor("emb", [GC * ni, H2], F32, kind="ExternalInput").ap()
    # host-prepped dense layouts (partition-major)
    wre = nc.dram_tensor("wre", [P, NB, ni], F32, kind="ExternalInput").ap()
    wt = nc.dram_tensor("wt", [P, 2, H2], F32, kind="ExternalInput").ap()
    bvec = nc.dram_tensor("bvec", [P, 2], F32, kind="ExternalInput").ap()
    ident = nc.dram_tensor("ident", [P, P], F32, kind="ExternalInput").ap()
    ents = nc.dram_tensor("ents", [NUM_ENTS, H], F32, kind="ExternalInput").ap()
    stem = nc.dram_tensor("stem", [BC, 1], mybir.dt.int32, kind="ExternalInput").ap()
    out = nc.dram_tensor("out", [GC, OUT_F], F32, kind="ExternalOutput").ap()

    # DRAM views
    e4 = emb.rearrange("(b p n) f -> b p n f", b=NB, p=P)        # [NB,128,ni,256]
    out_bp = out.rearrange("(b p) o -> b p o", b=NB)             # [NB,128,384]
    out_bt = out.rearrange("(bt s) o -> bt s o", s=SEQ)          # [64,10,384]

    with tile.TileContext(nc) as tc:
        with ExitStack() as ctx:
            consts = ctx.enter_context(tc.tile_pool(name="consts", bufs=1))
            ebuf = ctx.enter_context(tc.tile_pool(name="ebuf", bufs=6))
            pbuf = ctx.enter_context(tc.tile_pool(name="pbuf", bufs=6))
            small = ctx.enter_context(tc.tile_pool(name="small", bufs=1))
            blk = ctx.enter_context(tc.tile_pool(name="blk", bufs=2))
            psum_acc = ctx.enter_context(
                tc.tile_pool(name="psum_acc", bufs=2, space="PSUM"))
            psum_tr = ctx.enter_context(
                tc.tile_pool(name="psum_tr", bufs=2, space="PSUM"))
            psum_h = ctx.enter_context(
                tc.tile_pool(name="psum_h", bufs=2, space="PSUM"))

            # ---- pre-issue the first input DMAs so HBM streaming starts
            # before the consts issue/land ----
            pre = []
            if ni == NI:
                pre_chunks = [(k * 4, 4) for k in range(5)] + [(20, KI)]
            else:
                pre_chunks = [(k * KI, KI) for k in range(min(3, nch))]
            for r0, nr in pre_chunks:
                et0 = ebuf.tile([P, nr, H2], F32, name="et")
                nc.sync.dma_start(et0[:], e4[0, :, r0:r0 + nr, :])
                pre.append(et0)

            # ---- constants into SBUF (all dense, partition-major) ----
            ident_sb = consts.tile([P, P], F32)
            nc.sync.dma_start(ident_sb[:], ident[:, :])
            identr_sb = consts.tile([P, P], F32R)
            nc.vector.tensor_copy(identr_sb[:], ident_sb[:])
            wre_sb = consts.tile([P, NB, ni], F32)
            nc.sync.dma_start(wre_sb[:], wre[:, :, :])
            wt_sb = consts.tile([P, 2, H2], F32)
            nc.sync.dma_start(wt_sb[:], wt[:, :, :])
            wtr_sb = consts.tile([P, 2, H2], F32R)
            nc.vector.tensor_copy(wtr_sb[:], wt_sb[:])
            bias_sb = consts.tile([P, 2], F32)
            nc.sync.dma_start(bias_sb[:], bvec[:, :])
            stem_sb = consts.tile([BC, 1], mybir.dt.int32)
            nc.sync.dma_start(stem_sb[:], stem[:, :])

            # ---- phase 1 + per-block phase 2 ----
            for b in range(NB):
                acc = psum_acc.tile([P, 2 * H2], F32, name="acc")
                if ni == NI and b == NB - 1:
                    # taper the final chunks so the compute chain exposed
                    # after the very last DMA is short
                    chunk_list = [(k * KI, KI) for k in range(9)] + \
                                 [(9 * KI + k * 4, 4) for k in range(5)]
                elif ni == NI and b == 0:
                    # taper the opening chunks so the DMA queues fill and
                    # compute starts during the ramp
                    chunk_list = [(k * 4, 4) for k in range(5)] + \
                                 [(20 + k * KI, KI) for k in range(9)]
                else:
                    chunk_list = [(k * KI, KI) for k in range(nch)]
                n_chunks = len(chunk_list)
                for ci, (r0, nr) in enumerate(chunk_list):
                    if b == 0 and ci < len(pre):
                        et = pre[ci]
                    else:
                        et = ebuf.tile([P, nr, H2], F32, name="et")
                        nc.sync.dma_start(et[:], e4[b, :, r0:r0 + nr, :])
                    for i in range(0, nr, 2):
                        pair = pbuf.tile([P, 2, H2], F32R, name="pair")
                        for j in range(2):
                            k = r0 + i + j
                            nc.vector.tensor_scalar_mul(
                                pair[:, j, :], et[:, i + j, :],
                                wre_sb[:, b, k:k + 1],
                            )
                        nc.tensor.matmul(
                            acc[:],
                            lhsT=identr_sb[:],
                            rhs=pair[:, :, :],
                            start=(ci == 0 and i == 0),
                            stop=(ci == n_chunks - 1 and i == nr - 2),
                        )
                # fold the two halves: mean for this block [128, 256]
                mean_b = blk.tile([P, H2], F32, name="mean_b")
                nc.scalar.copy(mean_b[:], acc[:, 0:H2])
                nc.vector.tensor_add(mean_b[:], mean_b[:], acc[:, H2:2 * H2])

                # transpose to feature-major (float32r for the W matmul)
                meanT_b = blk.tile([P, 2, P], F32R, name="meanT_b")
                for fh in range(2):
                    pt = psum_tr.tile([P, P], F32, name="pt")
                    nc.tensor.transpose(
                        pt[:], mean_b[:, fh * P:(fh + 1) * P], ident_sb[:])
                    nc.scalar.copy(meanT_b[:, fh, :], pt[:])

                # hT = relu(WT.T @ meanT + bias)
                hT_b = blk.tile([P, 2, P], F32, name="hT_b")
                for oh in range(2):
                    ph = psum_h.tile([P, P], F32, name="ph")
                    for fh in range(2):
                        nc.tensor.matmul(
                            ph[:],
                            lhsT=wtr_sb[:, fh, oh * P:(oh + 1) * P],
                            rhs=meanT_b[:, fh, :],
                            start=(fh == 0),
                            stop=(fh == 1),
                        )
                    nc.scalar.activation(
                        hT_b[:, oh, :], ph[:],
                        func=mybir.ActivationFunctionType.Relu,
                        bias=bias_sb[:, oh:oh + 1],
                        scale=1.0,
                    )

                # back to group-major rows, store
                h_b = blk.tile([P, H2], F32, name="h_b")
                for oh in range(2):
                    pt2 = psum_tr.tile([P, P], F32, name="pt2")
                    nc.tensor.transpose(pt2[:], hT_b[:, oh, :], ident_sb[:])
                    nc.scalar.copy(h_b[:, oh * P:(oh + 1) * P], pt2[:])
                nc.sync.dma_start(out_bp[b, :, 0:H2], h_b[:])

                if b == 2:
                    # ent gather + broadcast store, scheduled mid-stream so
                    # it fills a block-boundary gap instead of the tail
                    ent_sb = small.tile([BC, H], F32)
                    nc.gpsimd.indirect_dma_start(
                        out=ent_sb[:],
                        out_offset=None,
                        in_=ents[:, :],
                        in_offset=bass.IndirectOffsetOnAxis(
                            ap=stem_sb[:, :1], axis=0),
                    )
                    ent_bc = bass.AP(
                        ent_sb.tensor, ent_sb.offset,
                        [ent_sb.ap[0], [0, SEQ], ent_sb.ap[1]],
                    )
                    nc.sync.dma_start(out_bt[:, :, H2:OUT_F], ent_bc)

    _split_multi_waits(nc)
    return nc


_PROGRAM_CACHE = {}


def _get_program(nch):
    if nch not in _PROGRAM_CACHE:
        _PROGRAM_CACHE[nch] = build_program(nch)
    return _PROGRAM_CACHE[nch]


def make_in_maps(embeds_stack, rel_weights, ent_embeds, W, b, length, s_tem,
                 nch=NI // KI):
    ni = nch * KI
    tot_c = GC * ni  # rows per core
    emb = np.ascontiguousarray(np.asarray(embeds_stack, dtype=np.float32))
    relw = np.asarray(rel_weights, dtype=np.float32)
    length = np.asarray(length, dtype=np.float32).reshape(NCORES * GC, 1)
    wmat = np.asarray(W, dtype=np.float32)
    bv = np.asarray(b, dtype=np.float32)
    ents = np.ascontiguousarray(np.asarray(ent_embeds, dtype=np.float32))
    stem = np.asarray(s_tem, dtype=np.int32)

    # wt[p, fh, o] = W[o, fh*128 + p]  (dense partition-major W^T chunks)
    wt = np.ascontiguousarray(
        wmat.T.reshape(2, P, H2).transpose(1, 0, 2))
    bvec = np.ascontiguousarray(bv.reshape(2, P).T)
    ident = np.eye(P, dtype=np.float32)

    # weight / length, laid out [128, NB, ni] per core (partition-major)
    wre_all = (relw.reshape(NCORES * GC, ni) / length).astype(np.float32)
    wre_all = wre_all.reshape(NCORES, NB, P, ni)

    in_maps = []
    for c in range(NCORES):
        in_maps.append({
            "emb": np.ascontiguousarray(emb[c * tot_c:(c + 1) * tot_c]),
            "wre": np.ascontiguousarray(wre_all[c].transpose(1, 0, 2)),
            "wt": wt,
            "bvec": bvec,
            "ident": ident,
            "ents": ents,
            "stem": np.ascontiguousarray(stem[c * BC:(c + 1) * BC].reshape(BC, 1)),
        })
    return in_maps


def run_sharded(inputs, nch=NI // KI, trace=False):
    """inputs: full-input dict as from reference.setup_inputs().
    Returns (full_output [512, 10, 384], BassKernelResults)."""
    nc = _get_program(nch)
    in_maps = make_in_maps(
        inputs["embeds_stack"], inputs["rel_weights"], inputs["ent_embeds"],
        inputs["W"], inputs["b"], inputs["length"], inputs["s_tem"], nch=nch,
    )
    last_err = None
    for attempt in range(3):
        try:
            res = bass_utils.run_bass_kernel_spmd(
                nc, in_maps, core_ids=list(range(NCORES)), trace=trace,
            )
            break
        except Exception as e:  # transient NRT_EXEC_UNIT_UNRECOVERABLE etc.
            last_err = e
            if attempt == 2:
                raise
            import time
            time.sleep(10)
    parts = [res.results[c]["out"].reshape(BC, SEQ, OUT_F) for c in range(NCORES)]
    full = np.concatenate(parts, axis=0)
    return full, res


def kernel(embeds_stack, rel_weights, ent_embeds, W, b,
           length, segment_ids, len_non_zero, s_tem):
    full, _ = run_sharded({
        "embeds_stack": embeds_stack, "rel_weights": rel_weights,
        "ent_embeds": ent_embeds, "W": W, "b": b, "length": length,
        "segment_ids": segment_ids, "len_non_zero": len_non_zero,
        "s_tem": s_tem,
    })
    return full
